# revision 5
# baseline (speedup 1.0000x reference)
"""GIN message-passing kernel for 8 TRN2 NeuronCores.

Nodes are sharded across 8 cores (6272 slots each, 49 tiles of 128). Edges are
partitioned by destination tile; source rows are fetched per edge with
gpsimd.dma_gather from a replicated fp32 table (x for layer 1, AllGather'ed h1
for layer 2). The gather is the hard bottleneck (~9.5 ns/row, HBM-latency
bound), so everything else is arranged to hide behind it:

- Self-edges are NOT gathered: the (1+eps)*x_i term is added with one identity
  matmul per tile from a per-core x_self input (layer 1) or from the SBUF-
  resident transposed h1 tiles (layer 2).
- Gather padding rows are never fetched: idx lists are padded with trailing -1
  and num_idxs_reg = per-(tile,half) max real count (host equalizes counts
  across cores with masked duplicate edges). G buffers are memset once so the
  unwritten tail columns stay finite (they are masked by the one-hot M).
- Table rows are laid out so tiles 0-31 map to rows [0, 32768) and tiles
  32-48 to [32768, 50176): the int16 gather index split doubles as the
  AllGather split. AG_A (tiles 0-31) overlaps layer-1 tail compute; AG_B
  overlaps the first layer-2 lo-gathers (hi-gathers lag by LAG tiles).
- Compute is bf16 (G cast on ACT, one-hot M built bf16 on DVE, bf16 MLP
  weights); PSUM accumulation stays fp32.
"""
import warnings

warnings.filterwarnings("ignore")

import numpy as np

N = 50000
E = 800000
F = 128
H = 128
C = 40
BN_EPS = 1e-5
NCORES = 8
P = 128
NT = 49              # tiles per core
NPC = NT * P         # 6272 node slots per core
NPAD = NCORES * NPC  # 50176
TLO = 32             # tiles 0..31 -> low rows
ROWS_LO = NCORES * TLO * P        # 32768 == int16 gather limit
ROWS_HI = NPAD - ROWS_LO          # 17408
LAG = 6              # layer-2 hi-gather lag (hides AG_B)


# ----------------------------------------------------------------- host prep

def _assign_nodes(deg):
    """Greedy balanced assignment of nodes to (core, tile, slot) by degree.

    Returns gid_of_orig[N]: gid = c*NPC + t*P + s.
    """
    order = np.argsort(-deg, kind="stable")
    core_load = np.zeros(NCORES, np.int64)
    core_cnt = np.zeros(NCORES, np.int64)
    node_core = np.empty(N, np.int32)
    for n in order:
        c = -1
        best = None
        for cc in range(NCORES):
            if core_cnt[cc] >= NPC:
                continue
            if best is None or core_load[cc] < best:
                best = core_load[cc]
                c = cc
        node_core[n] = c
        core_load[c] += deg[n]
        core_cnt[c] += 1

    gid_of_orig = np.empty(N, np.int64)
    for c in range(NCORES):
        nodes = order[node_core[order] == c]
        tile_load = np.zeros(NT, np.int64)
        tile_cnt = np.zeros(NT, np.int64)
        tl = np.empty(len(nodes), np.int32)
        for i, n in enumerate(nodes):
            avail = tile_cnt < P
            t = np.where(avail, tile_load, np.iinfo(np.int64).max).argmin()
            tl[i] = t
            tile_load[t] += deg[n]
            tile_cnt[t] += 1
        slot = np.zeros(NT, np.int64)
        for i, n in enumerate(nodes):
            t = tl[i]
            gid_of_orig[n] = c * NPC + t * P + slot[t]
            slot[t] += 1
    return gid_of_orig


def _assign_cells(node_ids, lo, hi, ncells, cap):
    """Greedy 2-D balance of nodes into ncells cells of capacity cap.

    Minimizes the per-cell max of normalized (lo, hi) loads. Returns cell id
    per node (aligned with node_ids order).
    """
    tot_lo, tot_hi = max(lo.sum(), 1), max(hi.sum(), 1)
    t_lo = tot_lo / ncells
    t_hi = tot_hi / ncells
    loads = np.zeros((ncells, 2), np.float64)
    cnt = np.zeros(ncells, np.int64)
    order = np.argsort(-(np.maximum(lo / t_lo, hi / t_hi)), kind="stable")
    cell_of = np.empty(len(node_ids), np.int64)
    for i in order:
        nl = (loads[:, 0] + lo[i]) / t_lo
        nh = (loads[:, 1] + hi[i]) / t_hi
        score = np.maximum(nl, nh)
        score[cnt >= cap] = np.inf
        cbest = int(np.argmin(score))
        cell_of[i] = cbest
        loads[cbest, 0] += lo[i]
        loads[cbest, 1] += hi[i]
        cnt[cbest] += 1
    return cell_of


def _row_of_gid(gid):
    """Table row for gid: tiles<TLO pack into [0, ROWS_LO), rest above."""
    c = gid // NPC
    r = gid % NPC
    t = r // P
    lo = t < TLO
    return np.where(lo, c * (TLO * P) + r, ROWS_LO + c * ((NT - TLO) * P)
                    + (r - TLO * P))


def _wrap_idx(idx):
    """[n] int -> [128, n//16] int16: idx i at [i%16, i//16], replicated x8."""
    n = len(idx)
    w = np.asarray(idx, np.int16).reshape(n // 16, 16).T
    return np.tile(w, (8, 1))


def _pack_edges(src_row, dst_gid):
    """Partition edges by (core, tile), split lo/hi by source row, equalize
    counts across cores with masked duplicates, pad idx to chunk multiples
    with -1 (not fetched).

    Returns (CH_LO, CH_HI, REG_LO, REG_HI, idx_pack, off_pack).
    """
    core = dst_gid // NPC
    tile = (dst_gid % NPC) // P
    off = dst_gid % P
    is_lo = src_row < ROWS_LO

    lists = [[None] * NT for _ in range(NCORES)]
    key = (core * NT + tile).astype(np.int64)
    order = np.argsort(key, kind="stable")
    rows_s, off_s, lo_s, key_s = (src_row[order], off[order], is_lo[order],
                                  key[order])
    bounds = np.searchsorted(key_s, np.arange(NCORES * NT + 1))
    for c in range(NCORES):
        for t in range(NT):
            b0, b1 = bounds[c * NT + t], bounds[c * NT + t + 1]
            m = lo_s[b0:b1]
            lists[c][t] = (
                (rows_s[b0:b1][m], off_s[b0:b1][m]),
                (rows_s[b0:b1][~m] - ROWS_LO, off_s[b0:b1][~m]),
            )

    def _dedup(rows, offs):
        """Pair up repeated source rows: each slot covers up to 2 edges."""
        if len(rows) == 0:
            return (rows, offs, np.full(0, P, np.float32))
        o = np.argsort(rows, kind="stable")
        r, f = rows[o], offs[o]
        # rank of each edge within its row group
        grp_start = np.r_[True, r[1:] != r[:-1]]
        gidx = np.cumsum(grp_start) - 1
        first_pos = np.flatnonzero(grp_start)[gidx]
        rank = np.arange(len(r)) - first_pos
        sel_a = (rank % 2) == 0
        ra, fa = r[sel_a], f[sel_a]
        fb = np.full(len(ra), P, np.float32)
        has_b = np.flatnonzero(sel_a)
        nb = has_b + 1 < len(r)
        pair_ok = np.zeros(len(ra), bool)
        pair_ok[nb] = (r[has_b[nb] + 1] == ra[nb]) & (rank[has_b[nb] + 1] % 2 == 1)
        fb[pair_ok] = f[has_b[pair_ok] + 1]
        return (ra, fa.astype(np.float32), fb)

    dedup = [[(_dedup(*lists[c][t][0]), _dedup(*lists[c][t][1]))
              for t in range(NT)] for c in range(NCORES)]
    cnt_lo = np.array([[len(dedup[c][t][0][0]) for t in range(NT)]
                       for c in range(NCORES)])
    cnt_hi = np.array([[len(dedup[c][t][1][0]) for t in range(NT)]
                       for c in range(NCORES)])
    REG_LO = cnt_lo.max(axis=0)          # per-tile max over cores
    REG_HI = cnt_hi.max(axis=0)
    REG_LO = np.maximum(REG_LO, 16)
    REG_HI = np.maximum(REG_HI, 16)
    CH_LO = ((REG_LO + P - 1) // P).astype(np.int64)
    CH_HI = ((REG_HI + P - 1) // P).astype(np.int64)

    idx_pack, offa_pack, offb_pack = [], [], []
    for c in range(NCORES):
        idx_cols, offa_cols, offb_cols = [], [], []
        for t in range(NT):
            for (rows_u, offa, offb), reg, chn in (
                (dedup[c][t][0], int(REG_LO[t]), int(CH_LO[t])),
                (dedup[c][t][1], int(REG_HI[t]), int(CH_HI[t])),
            ):
                nslots = chn * P
                nreal = len(rows_u)
                li = np.full(nslots, -1, np.int64)
                la = np.full(nslots, P, np.float32)
                lb = np.full(nslots, P, np.float32)
                li[:nreal] = rows_u
                la[:nreal] = offa
                lb[:nreal] = offb
                # masked duplicates up to reg so every core fetches exactly
                # reg rows (same compile-time num_idxs_reg for all cores)
                pad = reg - nreal
                if pad > 0:
                    if nreal > 0:
                        rep = np.resize(rows_u, pad)
                    else:
                        rep = (np.arange(pad, dtype=np.int64) * 97) % 1024
                    li[nreal:reg] = rep
                idx_cols.append(_wrap_idx(li))
                offa_cols.append(la.reshape(chn, P).T.astype(np.float32))
                offb_cols.append(lb.reshape(chn, P).T.astype(np.float32))
        idx_pack.append(np.ascontiguousarray(np.concatenate(idx_cols, axis=1)))
        offa_pack.append(np.ascontiguousarray(np.concatenate(offa_cols, axis=1)))
        offb_pack.append(np.ascontiguousarray(np.concatenate(offb_cols, axis=1)))
    return CH_LO, CH_HI, REG_LO, REG_HI, idx_pack, offa_pack, offb_pack


def _bf16(a):
    import ml_dtypes
    return np.ascontiguousarray(np.asarray(a, np.float32).astype(
        ml_dtypes.bfloat16))


def prepare(x, edge_index, W1a, bn_gamma, bn_beta, bn_mean, bn_var, W1b, W2a, W2b):
    x = np.asarray(x, np.float32)
    ei = np.asarray(edge_index, np.int64)
    src_o, dst_o = ei[0], ei[1]

    deg = np.bincount(dst_o, minlength=N).astype(np.int64)
    gid1 = _assign_nodes(deg)                      # phase 1: defines bands
    band_lo = ((gid1 % NPC) // P) < TLO            # per node, padded later
    # per-node lo/hi in-degree (by source band) -- stable under phase 2
    src_lo = band_lo[src_o]
    lo_in = np.bincount(dst_o[src_lo], minlength=N).astype(np.int64)
    hi_in = deg - lo_in

    # phase 2: rebalance within each band across all (core, tile) cells
    gid_of_orig = np.empty(N, np.int64)
    for in_band, tset in ((band_lo, np.arange(TLO)),
                          (~band_lo, np.arange(TLO, NT))):
        nodes = np.flatnonzero(in_band)
        ncells = NCORES * len(tset)
        cell_of = _assign_cells(nodes, lo_in[nodes].astype(np.float64),
                                hi_in[nodes].astype(np.float64), ncells, P)
        # cell k -> (core, tile): tiles of this band enumerated per core
        for k in np.unique(cell_of):
            members = nodes[cell_of == k]
            c = k // len(tset)
            t = tset[k % len(tset)]
            base = c * NPC + t * P
            gid_of_orig[members] = base + np.arange(len(members))
    row_of_gid = _row_of_gid(np.arange(NPAD, dtype=np.int64))

    src_row = row_of_gid[gid_of_orig[src_o]]
    dst_gid = gid_of_orig[dst_o]
    CH_LO, CH_HI, REG_LO, REG_HI, idx_pack, offa_pack, offb_pack = \
        _pack_edges(src_row, dst_gid)

    # x table in row order (fp32)
    x_pad = np.zeros((NPAD, F), np.float32)
    x_pad[row_of_gid[gid_of_orig]] = x

    # per-core self rows in (tile, slot) order
    x_gid = np.zeros((NPAD, F), np.float32)
    x_gid[gid_of_orig] = x

    scale = (np.asarray(bn_gamma) / np.sqrt(np.asarray(bn_var) + BN_EPS)
             ).astype(np.float32)
    bias = (np.asarray(bn_beta) - np.asarray(bn_mean) * scale).astype(
        np.float32)

    consts = {
        "x_pad": x_pad,
        "W1aT": _bf16(np.asarray(W1a, np.float32).T),
        "W1bT": _bf16(np.asarray(W1b, np.float32).T),
        "W2aT": _bf16(np.asarray(W2a, np.float32).T),
        "W2bT": _bf16(np.asarray(W2b, np.float32).T),
        "bn_s": scale.reshape(H, 1),
        "bn_b": bias.reshape(H, 1),
        "iota": _bf16(np.tile(np.arange(P, dtype=np.float32), (P, 1))),
    }
    in_maps = []
    for c in range(NCORES):
        m = dict(consts)
        m["idx_all"] = idx_pack[c]
        m["offa_all"] = _bf16(offa_pack[c])
        m["offb_all"] = _bf16(offb_pack[c])
        m["x_self"] = _bf16(x_gid[c * NPC:(c + 1) * NPC])
        in_maps.append(m)
    key_a = np.concatenate([CH_LO, REG_LO])
    key_b = np.concatenate([CH_HI, REG_HI])
    return in_maps, key_a, key_b, gid_of_orig


# -------------------------------------------------------------- bass program

def build(key_a, key_b, do_gather=True, do_compute=True, do_cc=True,
          nqueues=1, single_packet=False):
    import concourse.bacc as bacc
    import concourse.mybir as mybir
    import concourse.tile as tile
    from concourse.masks import make_identity

    CH_LO, REG_LO = key_a[:NT].astype(np.int64), key_a[NT:].astype(np.int64)
    CH_HI, REG_HI = key_b[:NT].astype(np.int64), key_b[NT:].astype(np.int64)

    nc = bacc.Bacc("TRN2", target_bir_lowering=False, debug=False,
                   num_devices=NCORES, num_swdge_queues=nqueues)
    f32 = mybir.dt.float32
    bf16 = mybir.dt.bfloat16
    
    CH = CH_LO + CH_HI
    CH_TOT = int(CH.sum())
    CH_MAX = int(CH.max())
    S_TOT = int(8 * CH_TOT)

    x_pad = nc.dram_tensor("x_pad", [NPAD, F], f32, kind="ExternalInput")
    x_self = nc.dram_tensor("x_self", [NPC, F], bf16, kind="ExternalInput")
    idx_all = nc.dram_tensor("idx_all", [P, S_TOT], mybir.dt.int16,
                             kind="ExternalInput")
    offa_all = nc.dram_tensor("offa_all", [P, CH_TOT], bf16, kind="ExternalInput")
    offb_all = nc.dram_tensor("offb_all", [P, CH_TOT], bf16, kind="ExternalInput")
    W1aT = nc.dram_tensor("W1aT", [F, H], bf16, kind="ExternalInput")
    W1bT = nc.dram_tensor("W1bT", [H, H], bf16, kind="ExternalInput")
    W2aT = nc.dram_tensor("W2aT", [H, H], bf16, kind="ExternalInput")
    W2bT = nc.dram_tensor("W2bT", [H, C], bf16, kind="ExternalInput")
    bn_s = nc.dram_tensor("bn_s", [H, 1], f32, kind="ExternalInput")
    bn_b = nc.dram_tensor("bn_b", [H, 1], f32, kind="ExternalInput")
    iota = nc.dram_tensor("iota", [P, P], bf16, kind="ExternalInput")
    outT = nc.dram_tensor("outT", [C, NPC], f32, kind="ExternalOutput")

    Relu = mybir.ActivationFunctionType.Relu
    Copy = mybir.ActivationFunctionType.Copy

    with tile.TileContext(nc) as tc:
        with (
            tc.tile_pool(name="const", bufs=1) as cst,
            tc.tile_pool(name="gbuf", bufs=8) as gp,
            tc.tile_pool(name="gbbuf", bufs=3) as gbp,
            tc.tile_pool(name="mbuf", bufs=3) as mp,
            tc.tile_pool(name="small", bufs=3) as sp,
            tc.tile_pool(name="ps_agg", bufs=2, space="PSUM") as ps_agg,
            tc.tile_pool(name="ps_t", bufs=2, space="PSUM") as ps_t,
            tc.tile_pool(name="ps_mm", bufs=2, space="PSUM") as ps_mm,
            tc.tile_pool(name="dram", bufs=1, space="DRAM") as dram,
        ):
            ident = cst.tile([P, P], f32)
            make_identity(nc, ident[:])
            identb = cst.tile([P, P], bf16)
            nc.scalar.activation(out=identb[:], in_=ident[:], func=Copy)
            iota_sb = cst.tile([P, P], bf16)
            nc.sync.dma_start(out=iota_sb[:], in_=iota[:])
            w1a_sb = cst.tile([F, H], bf16)
            nc.sync.dma_start(out=w1a_sb[:], in_=W1aT[:])
            w1b_sb = cst.tile([H, H], bf16)
            nc.sync.dma_start(out=w1b_sb[:], in_=W1bT[:])
            w2a_sb = cst.tile([H, H], bf16)
            nc.sync.dma_start(out=w2a_sb[:], in_=W2aT[:])
            w2b_sb = cst.tile([H, C], bf16)
            nc.sync.dma_start(out=w2b_sb[:], in_=W2bT[:])
            bns_sb = cst.tile([H, 1], f32)
            nc.sync.dma_start(out=bns_sb[:], in_=bn_s[:])
            bnb_sb = cst.tile([H, 1], f32)
            nc.sync.dma_start(out=bnb_sb[:], in_=bn_b[:])
            idx_sb = cst.tile([P, S_TOT], mybir.dt.int16)
            nc.sync.dma_start(out=idx_sb[:], in_=idx_all[:])
            offa_sb = cst.tile([P, CH_TOT], bf16)
            nc.sync.dma_start(out=offa_sb[:], in_=offa_all[:])
            offb_sb = cst.tile([P, CH_TOT], bf16)
            nc.sync.dma_start(out=offb_sb[:], in_=offb_all[:])
            h1keep = cst.tile([P, NT * P], bf16)   # transposed h1, bf16

            # initialize G ring so masked (unfetched) columns stay finite
            for _ in range(8):
                Gz = gp.tile([P, CH_MAX, F], f32, tag="G")
                nc.vector.memset(Gz[:], 0.0)

            slice_a = dram.tile([TLO * P, H], f32)
            slice_b = dram.tile([(NT - TLO) * P, H], f32)
            full_a = dram.tile([ROWS_LO, H], f32)
            full_b = dram.tile([ROWS_HI, H], f32)

            icol = np.concatenate([[0], np.cumsum((CH_LO + CH_HI) * 8)])
            ocol = np.concatenate([[0], np.cumsum(CH_LO + CH_HI)])

            def gather_lo(t, tab_lo, G):
                chl = int(CH_LO[t])
                ic = int(icol[t])
                if do_gather:
                    nc.gpsimd.dma_gather(
                        G[:, 0:chl, :], tab_lo, idx_sb[:, ic:ic + chl * 8],
                        chl * P, int(REG_LO[t]), F, single_packet=single_packet,
                        queue_num=(2 * t) % nqueues)

            def gather_hi(t, tab_hi, G):
                chl, chh = int(CH_LO[t]), int(CH_HI[t])
                ic = int(icol[t])
                if do_gather:
                    nc.gpsimd.dma_gather(
                        G[:, chl:chl + chh, :], tab_hi,
                        idx_sb[:, ic + chl * 8:ic + (chl + chh) * 8],
                        chh * P, int(REG_HI[t]), F, single_packet=single_packet,
                        queue_num=(2 * t + 1) % nqueues)

            def aggregate(t, G, self_sb):
                """one-hot segment-sum of G plus self term -> agg_sb bf16."""
                ch = int(CH[t])
                Gb = gbp.tile([P, CH_MAX, F], bf16, tag="Gb")
                nc.scalar.activation(out=Gb[:, :ch, :], in_=G[:, :ch, :],
                                     func=Copy)
                M = mp.tile([P, CH_MAX * P], bf16, tag="M")
                Mb = mp.tile([P, CH_MAX * P], bf16, tag="Mb")
                oc = int(ocol[t])
                nc.vector.tensor_tensor(
                    out=M[:, :ch * P],
                    in0=offa_sb[:, oc:oc + ch, None].to_broadcast([P, ch, P]),
                    in1=iota_sb[:, None, :].to_broadcast([P, ch, P]),
                    op=mybir.AluOpType.is_equal,
                )
                nc.vector.tensor_tensor(
                    out=Mb[:, :ch * P],
                    in0=offb_sb[:, oc:oc + ch, None].to_broadcast([P, ch, P]),
                    in1=iota_sb[:, None, :].to_broadcast([P, ch, P]),
                    op=mybir.AluOpType.is_equal,
                )
                nc.vector.tensor_tensor(
                    out=M[:, :ch * P], in0=M[:, :ch * P], in1=Mb[:, :ch * P],
                    op=mybir.AluOpType.add,
                )
                agg_ps = ps_agg.tile([F, P], f32, tag="agg")
                for k in range(ch):
                    nc.tensor.matmul(out=agg_ps[:], lhsT=Gb[:, k, :],
                                     rhs=M[:, k * P:(k + 1) * P],
                                     start=(k == 0), stop=False)
                nc.tensor.matmul(out=agg_ps[:], lhsT=self_sb, rhs=identb[:],
                                 start=False, stop=True)
                agg_sb = sp.tile([F, P], bf16, tag="agg_sb")
                nc.scalar.activation(out=agg_sb[:], in_=agg_ps[:], func=Copy)
                return agg_sb

            # ---------------- layer 1 ----------------
            for t in range(NT):
                G = gp.tile([P, CH_MAX, F], f32, tag="G")
                gather_lo(t, x_pad[0:ROWS_LO, :], G)
                gather_hi(t, x_pad[ROWS_LO:NPAD, :], G)
                if not do_compute:
                    continue
                xs = sp.tile([P, F], bf16, tag="xs")
                nc.sync.dma_start(
                    out=xs[:], in_=x_self[t * P:(t + 1) * P, :])
                agg_sb = aggregate(t, G, xs[:])
                h1a_ps = ps_mm.tile([H, P], f32, tag="mma")
                nc.tensor.matmul(out=h1a_ps[:], lhsT=w1a_sb[:], rhs=agg_sb[:],
                                 start=True, stop=True)
                h1a_sb = sp.tile([H, P], bf16, tag="h1a")
                nc.scalar.activation(out=h1a_sb[:], in_=h1a_ps[:], func=Relu,
                                     bias=bnb_sb[:, :1], scale=bns_sb[:, :1])
                h1b_ps = ps_mm.tile([H, P], f32, tag="mmb")
                nc.tensor.matmul(out=h1b_ps[:], lhsT=w1b_sb[:], rhs=h1a_sb[:],
                                 start=True, stop=True)
                h1b_sb = sp.tile([H, P], f32, tag="h1b")
                nc.scalar.activation(out=h1b_sb[:], in_=h1b_ps[:], func=Relu)
                ht_ps = ps_t.tile([P, H], f32, tag="trans")
                nc.tensor.transpose(out=ht_ps[:], in_=h1b_sb[:],
                                    identity=ident[:])
                ht_sb = sp.tile([P, H], f32, tag="ht")
                nc.scalar.activation(out=ht_sb[:], in_=ht_ps[:], func=Copy)
                nc.vector.tensor_copy(out=h1keep[:, t * P:(t + 1) * P],
                                      in_=ht_ps[:])
                if t < TLO:
                    nc.sync.dma_start(
                        out=slice_a[t * P:(t + 1) * P, :], in_=ht_sb[:])
                else:
                    tt = t - TLO
                    nc.sync.dma_start(
                        out=slice_b[tt * P:(tt + 1) * P, :], in_=ht_sb[:])
            if do_cc and do_compute:
                # AG_A is issued after the loop but depends only on slice_a
                # (tiles 0-31); the Tile scheduler runs it during the layer-1
                # tail. AG_B was issued above at t == NT-1.
                nc.gpsimd.collective_compute(
                    "AllGather", mybir.AluOpType.bypass,
                    replica_groups=[list(range(NCORES))],
                    ins=[slice_a.opt()], outs=[full_a.opt()],
                )

            # ---------------- layer 2 ----------------
            # hi-gathers lag LAG tiles behind lo-gathers so AG_B hides
            glist = [None] * NT

            def l2_compute(t):
                G = glist[t]
                agg_sb = aggregate(t, G, h1keep[:, t * P:(t + 1) * P])
                h2_ps = ps_mm.tile([H, P], f32, tag="mma")
                nc.tensor.matmul(out=h2_ps[:], lhsT=w2a_sb[:], rhs=agg_sb[:],
                                 start=True, stop=True)
                h2_sb = sp.tile([H, P], bf16, tag="h1a")
                nc.scalar.activation(out=h2_sb[:], in_=h2_ps[:], func=Relu)
                o_ps = ps_mm.tile([C, P], f32, tag="mmb")
                nc.tensor.matmul(out=o_ps[:], lhsT=w2b_sb[:], rhs=h2_sb[:],
                                 start=True, stop=True)
                o_sb = sp.tile([C, P], f32, tag="out")
                nc.scalar.activation(out=o_sb[:], in_=o_ps[:], func=Relu)
                nc.sync.dma_start(out=outT[:, t * P:(t + 1) * P], in_=o_sb[:])

            for t in range(NT):
                G = gp.tile([P, CH_MAX, F], f32, tag="G")
                glist[t] = G
                gather_lo(t, full_a[:], G)
                if t == LAG and do_cc and do_compute:
                    nc.gpsimd.collective_compute(
                        "AllGather", mybir.AluOpType.bypass,
                        replica_groups=[list(range(NCORES))],
                        ins=[slice_b.opt()], outs=[full_b.opt()],
                    )
                if t >= LAG:
                    th = t - LAG
                    gather_hi(th, full_b[:], glist[th])
                    if do_compute:
                        l2_compute(th)
            for th in range(NT - LAG, NT):
                gather_hi(th, full_b[:], glist[th])
                if do_compute:
                    l2_compute(th)

    nc.compile()
    return nc


# ------------------------------------------------------------------- driver

_CACHE = {}


def kernel(x, edge_index, W1a, bn_gamma, bn_beta, bn_mean, bn_var, W1b, W2a, W2b,
           _trace=False):
    from concourse.bass_utils import run_bass_kernel_spmd

    in_maps, key_a, key_b, gid_of_orig = prepare(
        x, edge_index, W1a, bn_gamma, bn_beta, bn_mean, bn_var, W1b, W2a, W2b
    )
    key = (tuple(key_a), tuple(key_b))
    if key not in _CACHE:
        _CACHE[key] = build(key_a, key_b)
    nc = _CACHE[key]

    res = run_bass_kernel_spmd(nc, in_maps, core_ids=list(range(NCORES)))
    outT = np.concatenate([r["outT"] for r in res.results], axis=1)  # [C, NPAD]
    out = outT.T[gid_of_orig]  # [N, C]
    if _trace:
        kernel.last_results = res
    return np.ascontiguousarray(out.astype(np.float32))



# revision 6
# speedup vs baseline: 1.2090x; 1.2090x over previous
"""GIN message-passing kernel for 8 TRN2 NeuronCores.

Nodes are sharded across 8 cores (6272 slots each, 49 tiles of 128). Edges are
partitioned by destination tile; source rows are fetched per edge with
gpsimd.dma_gather from a replicated fp32 table (x for layer 1, AllGather'ed h1
for layer 2). The gather is the hard bottleneck (~9.5 ns/row, HBM-latency
bound), so everything else is arranged to hide behind it:

- Self-edges are NOT gathered: the (1+eps)*x_i term is added with one identity
  matmul per tile from a per-core x_self input (layer 1) or from the SBUF-
  resident transposed h1 tiles (layer 2).
- Gather padding rows are never fetched: idx lists are padded with trailing -1
  and num_idxs_reg = per-(tile,half) max real count (host equalizes counts
  across cores with masked duplicate edges). G buffers are memset once so the
  unwritten tail columns stay finite (they are masked by the one-hot M).
- Table rows are laid out so tiles 0-31 map to rows [0, 32768) and tiles
  32-48 to [32768, 50176): the int16 gather index split doubles as the
  AllGather split. AG_A (tiles 0-31) overlaps layer-1 tail compute; AG_B
  overlaps the first layer-2 lo-gathers (hi-gathers lag by LAG tiles).
- Compute is bf16 (G cast on ACT, one-hot M built bf16 on DVE, bf16 MLP
  weights); PSUM accumulation stays fp32.
"""
import warnings

warnings.filterwarnings("ignore")

import numpy as np

N = 50000
E = 800000
F = 128
H = 128
C = 40
BN_EPS = 1e-5
NCORES = 8
P = 128
NT = 49              # tiles per core
NPC = NT * P         # 6272 node slots per core
NPAD = NCORES * NPC  # 50176
TLO = 32             # tiles 0..31 -> low rows
ROWS_LO = NCORES * TLO * P        # 32768 == int16 gather limit
ROWS_HI = NPAD - ROWS_LO          # 17408
LAG = 6              # layer-2 hi-gather lag (hides AG_B)


# ----------------------------------------------------------------- host prep

def _assign_nodes(deg):
    """Greedy balanced assignment of nodes to (core, tile, slot) by degree.

    Returns gid_of_orig[N]: gid = c*NPC + t*P + s.
    """
    order = np.argsort(-deg, kind="stable")
    core_load = np.zeros(NCORES, np.int64)
    core_cnt = np.zeros(NCORES, np.int64)
    node_core = np.empty(N, np.int32)
    for n in order:
        c = -1
        best = None
        for cc in range(NCORES):
            if core_cnt[cc] >= NPC:
                continue
            if best is None or core_load[cc] < best:
                best = core_load[cc]
                c = cc
        node_core[n] = c
        core_load[c] += deg[n]
        core_cnt[c] += 1

    gid_of_orig = np.empty(N, np.int64)
    for c in range(NCORES):
        nodes = order[node_core[order] == c]
        tile_load = np.zeros(NT, np.int64)
        tile_cnt = np.zeros(NT, np.int64)
        tl = np.empty(len(nodes), np.int32)
        for i, n in enumerate(nodes):
            avail = tile_cnt < P
            t = np.where(avail, tile_load, np.iinfo(np.int64).max).argmin()
            tl[i] = t
            tile_load[t] += deg[n]
            tile_cnt[t] += 1
        slot = np.zeros(NT, np.int64)
        for i, n in enumerate(nodes):
            t = tl[i]
            gid_of_orig[n] = c * NPC + t * P + slot[t]
            slot[t] += 1
    return gid_of_orig


def _assign_cells(node_ids, lo, hi, ncells, cap):
    """Greedy 2-D balance of nodes into ncells cells of capacity cap.

    Minimizes the per-cell max of normalized (lo, hi) loads. Returns cell id
    per node (aligned with node_ids order).
    """
    tot_lo, tot_hi = max(lo.sum(), 1), max(hi.sum(), 1)
    t_lo = tot_lo / ncells
    t_hi = tot_hi / ncells
    loads = np.zeros((ncells, 2), np.float64)
    cnt = np.zeros(ncells, np.int64)
    order = np.argsort(-(np.maximum(lo / t_lo, hi / t_hi)), kind="stable")
    cell_of = np.empty(len(node_ids), np.int64)
    for i in order:
        nl = (loads[:, 0] + lo[i]) / t_lo
        nh = (loads[:, 1] + hi[i]) / t_hi
        score = np.maximum(nl, nh)
        score[cnt >= cap] = np.inf
        cbest = int(np.argmin(score))
        cell_of[i] = cbest
        loads[cbest, 0] += lo[i]
        loads[cbest, 1] += hi[i]
        cnt[cbest] += 1
    return cell_of


def _row_of_gid(gid):
    """Table row for gid: tiles<TLO pack into [0, ROWS_LO), rest above."""
    c = gid // NPC
    r = gid % NPC
    t = r // P
    lo = t < TLO
    return np.where(lo, c * (TLO * P) + r, ROWS_LO + c * ((NT - TLO) * P)
                    + (r - TLO * P))


def _wrap_idx(idx):
    """[n] int -> [128, n//16] int16: idx i at [i%16, i//16], replicated x8."""
    n = len(idx)
    w = np.asarray(idx, np.int16).reshape(n // 16, 16).T
    return np.tile(w, (8, 1))


def _pack_edges(src_row, dst_gid):
    """Partition edges by (core, tile), split lo/hi by source row, equalize
    counts across cores with masked duplicates, pad idx to chunk multiples
    with -1 (not fetched).

    Returns (CH_LO, CH_HI, REG_LO, REG_HI, idx_pack, off_pack).
    """
    core = dst_gid // NPC
    tile = (dst_gid % NPC) // P
    off = dst_gid % P
    is_lo = src_row < ROWS_LO

    lists = [[None] * NT for _ in range(NCORES)]
    key = (core * NT + tile).astype(np.int64)
    order = np.argsort(key, kind="stable")
    rows_s, off_s, lo_s, key_s = (src_row[order], off[order], is_lo[order],
                                  key[order])
    bounds = np.searchsorted(key_s, np.arange(NCORES * NT + 1))
    for c in range(NCORES):
        for t in range(NT):
            b0, b1 = bounds[c * NT + t], bounds[c * NT + t + 1]
            m = lo_s[b0:b1]
            lists[c][t] = (
                (rows_s[b0:b1][m], off_s[b0:b1][m]),
                (rows_s[b0:b1][~m] - ROWS_LO, off_s[b0:b1][~m]),
            )

    def _dedup(rows, offs):
        """Pair up repeated source rows: each slot covers up to 2 edges."""
        if len(rows) == 0:
            return (rows, offs, np.full(0, P, np.float32))
        o = np.argsort(rows, kind="stable")
        r, f = rows[o], offs[o]
        # rank of each edge within its row group
        grp_start = np.r_[True, r[1:] != r[:-1]]
        gidx = np.cumsum(grp_start) - 1
        first_pos = np.flatnonzero(grp_start)[gidx]
        rank = np.arange(len(r)) - first_pos
        sel_a = (rank % 2) == 0
        ra, fa = r[sel_a], f[sel_a]
        fb = np.full(len(ra), P, np.float32)
        has_b = np.flatnonzero(sel_a)
        nb = has_b + 1 < len(r)
        pair_ok = np.zeros(len(ra), bool)
        pair_ok[nb] = (r[has_b[nb] + 1] == ra[nb]) & (rank[has_b[nb] + 1] % 2 == 1)
        fb[pair_ok] = f[has_b[pair_ok] + 1]
        return (ra, fa.astype(np.float32), fb)

    dedup = [[(_dedup(*lists[c][t][0]), _dedup(*lists[c][t][1]))
              for t in range(NT)] for c in range(NCORES)]
    cnt_lo = np.array([[len(dedup[c][t][0][0]) for t in range(NT)]
                       for c in range(NCORES)])
    cnt_hi = np.array([[len(dedup[c][t][1][0]) for t in range(NT)]
                       for c in range(NCORES)])
    REG_LO = cnt_lo.max(axis=0)          # per-tile max over cores
    REG_HI = cnt_hi.max(axis=0)
    REG_LO = np.maximum(REG_LO, 16)
    REG_HI = np.maximum(REG_HI, 16)
    CH_LO = ((REG_LO + P - 1) // P).astype(np.int64)
    CH_HI = ((REG_HI + P - 1) // P).astype(np.int64)

    idx_pack, offa_pack, offb_pack = [], [], []
    for c in range(NCORES):
        idx_cols, offa_cols, offb_cols = [], [], []
        for t in range(NT):
            for (rows_u, offa, offb), reg, chn in (
                (dedup[c][t][0], int(REG_LO[t]), int(CH_LO[t])),
                (dedup[c][t][1], int(REG_HI[t]), int(CH_HI[t])),
            ):
                nslots = chn * P
                nreal = len(rows_u)
                li = np.full(nslots, -1, np.int64)
                la = np.full(nslots, P, np.float32)
                lb = np.full(nslots, P, np.float32)
                li[:nreal] = rows_u
                la[:nreal] = offa
                lb[:nreal] = offb
                # masked duplicates up to reg so every core fetches exactly
                # reg rows (same compile-time num_idxs_reg for all cores)
                pad = reg - nreal
                if pad > 0:
                    if nreal > 0:
                        rep = np.resize(rows_u, pad)
                    else:
                        rep = (np.arange(pad, dtype=np.int64) * 97) % 1024
                    li[nreal:reg] = rep
                idx_cols.append(_wrap_idx(li))
                offa_cols.append(la.reshape(chn, P).T.astype(np.float32))
                offb_cols.append(lb.reshape(chn, P).T.astype(np.float32))
        idx_pack.append(np.ascontiguousarray(np.concatenate(idx_cols, axis=1)))
        offa_pack.append(np.ascontiguousarray(np.concatenate(offa_cols, axis=1)))
        offb_pack.append(np.ascontiguousarray(np.concatenate(offb_cols, axis=1)))
    return CH_LO, CH_HI, REG_LO, REG_HI, idx_pack, offa_pack, offb_pack


def _bf16(a):
    import ml_dtypes
    return np.ascontiguousarray(np.asarray(a, np.float32).astype(
        ml_dtypes.bfloat16))


def prepare(x, edge_index, W1a, bn_gamma, bn_beta, bn_mean, bn_var, W1b, W2a, W2b):
    x = np.asarray(x, np.float32)
    ei = np.asarray(edge_index, np.int64)
    src_o, dst_o = ei[0], ei[1]

    deg = np.bincount(dst_o, minlength=N).astype(np.int64)
    gid1 = _assign_nodes(deg)                      # phase 1: defines bands
    band_lo = ((gid1 % NPC) // P) < TLO            # per node, padded later
    # per-node lo/hi in-degree (by source band) -- stable under phase 2
    src_lo = band_lo[src_o]
    lo_in = np.bincount(dst_o[src_lo], minlength=N).astype(np.int64)
    hi_in = deg - lo_in

    # phase 2: rebalance within each band across all (core, tile) cells
    gid_of_orig = np.empty(N, np.int64)
    for in_band, tset in ((band_lo, np.arange(TLO)),
                          (~band_lo, np.arange(TLO, NT))):
        nodes = np.flatnonzero(in_band)
        ncells = NCORES * len(tset)
        cell_of = _assign_cells(nodes, lo_in[nodes].astype(np.float64),
                                hi_in[nodes].astype(np.float64), ncells, P)
        # cell k -> (core, tile): tiles of this band enumerated per core
        for k in np.unique(cell_of):
            members = nodes[cell_of == k]
            c = k // len(tset)
            t = tset[k % len(tset)]
            base = c * NPC + t * P
            gid_of_orig[members] = base + np.arange(len(members))
    row_of_gid = _row_of_gid(np.arange(NPAD, dtype=np.int64))

    src_row = row_of_gid[gid_of_orig[src_o]]
    dst_gid = gid_of_orig[dst_o]
    CH_LO, CH_HI, REG_LO, REG_HI, idx_pack, offa_pack, offb_pack = \
        _pack_edges(src_row, dst_gid)

    # x table in row order (fp32)
    x_pad = np.zeros((NPAD, F), np.float32)
    x_pad[row_of_gid[gid_of_orig]] = x

    # per-core self rows in (tile, slot) order
    x_gid = np.zeros((NPAD, F), np.float32)
    x_gid[gid_of_orig] = x

    scale = (np.asarray(bn_gamma) / np.sqrt(np.asarray(bn_var) + BN_EPS)
             ).astype(np.float32)
    bias = (np.asarray(bn_beta) - np.asarray(bn_mean) * scale).astype(
        np.float32)

    consts = {
        "x_pad": x_pad,
        "W1aT": _bf16(np.asarray(W1a, np.float32).T),
        "W1bT": _bf16(np.asarray(W1b, np.float32).T),
        "W2aT": _bf16(np.asarray(W2a, np.float32).T),
        "W2bT": _bf16(np.asarray(W2b, np.float32).T),
        "bn_s": scale.reshape(H, 1),
        "bn_b": bias.reshape(H, 1),
        "iota": _bf16(np.tile(np.arange(P, dtype=np.float32), (P, 1))),
    }
    in_maps = []
    for c in range(NCORES):
        m = dict(consts)
        m["idx_all"] = idx_pack[c]
        m["offa_all"] = _bf16(offa_pack[c])
        m["offb_all"] = _bf16(offb_pack[c])
        m["x_self"] = _bf16(x_gid[c * NPC:(c + 1) * NPC])
        in_maps.append(m)
    key_a = np.concatenate([CH_LO, REG_LO])
    key_b = np.concatenate([CH_HI, REG_HI])
    return in_maps, key_a, key_b, gid_of_orig


# -------------------------------------------------------------- bass program

def build(key_a, key_b, do_gather=True, do_compute=True, do_cc=True,
          nqueues=1, single_packet=False):
    import concourse.bacc as bacc
    import concourse.mybir as mybir
    import concourse.tile as tile
    from concourse.masks import make_identity

    CH_LO, REG_LO = key_a[:NT].astype(np.int64), key_a[NT:].astype(np.int64)
    CH_HI, REG_HI = key_b[:NT].astype(np.int64), key_b[NT:].astype(np.int64)

    nc = bacc.Bacc("TRN2", target_bir_lowering=False, debug=False,
                   num_devices=NCORES, num_swdge_queues=nqueues)
    f32 = mybir.dt.float32
    bf16 = mybir.dt.bfloat16
    
    CH = CH_LO + CH_HI
    CH_TOT = int(CH.sum())
    CH_MAX = int(CH.max())
    S_TOT = int(8 * CH_TOT)

    x_pad = nc.dram_tensor("x_pad", [NPAD, F], f32, kind="ExternalInput")
    x_self = nc.dram_tensor("x_self", [NPC, F], bf16, kind="ExternalInput")
    idx_all = nc.dram_tensor("idx_all", [P, S_TOT], mybir.dt.int16,
                             kind="ExternalInput")
    offa_all = nc.dram_tensor("offa_all", [P, CH_TOT], bf16, kind="ExternalInput")
    offb_all = nc.dram_tensor("offb_all", [P, CH_TOT], bf16, kind="ExternalInput")
    W1aT = nc.dram_tensor("W1aT", [F, H], bf16, kind="ExternalInput")
    W1bT = nc.dram_tensor("W1bT", [H, H], bf16, kind="ExternalInput")
    W2aT = nc.dram_tensor("W2aT", [H, H], bf16, kind="ExternalInput")
    W2bT = nc.dram_tensor("W2bT", [H, C], bf16, kind="ExternalInput")
    bn_s = nc.dram_tensor("bn_s", [H, 1], f32, kind="ExternalInput")
    bn_b = nc.dram_tensor("bn_b", [H, 1], f32, kind="ExternalInput")
    iota = nc.dram_tensor("iota", [P, P], bf16, kind="ExternalInput")
    outT = nc.dram_tensor("outT", [C, NPC], f32, kind="ExternalOutput")

    Relu = mybir.ActivationFunctionType.Relu
    Copy = mybir.ActivationFunctionType.Copy

    with tile.TileContext(nc) as tc:
        with (
            tc.tile_pool(name="const", bufs=1) as cst,
            tc.tile_pool(name="gbuf", bufs=8) as gp,
            tc.tile_pool(name="gbbuf", bufs=3) as gbp,
            tc.tile_pool(name="mbuf", bufs=3) as mp,
            tc.tile_pool(name="small", bufs=3) as sp,
            tc.tile_pool(name="ps_agg", bufs=2, space="PSUM") as ps_agg,
            tc.tile_pool(name="ps_t", bufs=2, space="PSUM") as ps_t,
            tc.tile_pool(name="ps_mm", bufs=2, space="PSUM") as ps_mm,
            tc.tile_pool(name="dram", bufs=1, space="DRAM") as dram,
        ):
            ident = cst.tile([P, P], f32)
            make_identity(nc, ident[:])
            identb = cst.tile([P, P], bf16)
            nc.scalar.activation(out=identb[:], in_=ident[:], func=Copy)
            iota_sb = cst.tile([P, P], bf16)
            nc.sync.dma_start(out=iota_sb[:], in_=iota[:])
            w1a_sb = cst.tile([F, H], bf16)
            nc.sync.dma_start(out=w1a_sb[:], in_=W1aT[:])
            w1b_sb = cst.tile([H, H], bf16)
            nc.sync.dma_start(out=w1b_sb[:], in_=W1bT[:])
            w2a_sb = cst.tile([H, H], bf16)
            nc.sync.dma_start(out=w2a_sb[:], in_=W2aT[:])
            w2b_sb = cst.tile([H, C], bf16)
            nc.sync.dma_start(out=w2b_sb[:], in_=W2bT[:])
            bns_sb = cst.tile([H, 1], f32)
            nc.sync.dma_start(out=bns_sb[:], in_=bn_s[:])
            bnb_sb = cst.tile([H, 1], f32)
            nc.sync.dma_start(out=bnb_sb[:], in_=bn_b[:])
            idx_sb = cst.tile([P, S_TOT], mybir.dt.int16)
            nc.sync.dma_start(out=idx_sb[:], in_=idx_all[:])
            offa_sb = cst.tile([P, CH_TOT], bf16)
            nc.sync.dma_start(out=offa_sb[:], in_=offa_all[:])
            offb_sb = cst.tile([P, CH_TOT], bf16)
            nc.sync.dma_start(out=offb_sb[:], in_=offb_all[:])
            h1keep = cst.tile([P, NT * P], bf16)   # transposed h1, bf16

            # initialize G ring so masked (unfetched) columns stay finite
            for _ in range(8):
                Gz = gp.tile([P, CH_MAX, F], f32, tag="G")
                nc.vector.memset(Gz[:], 0.0)

            slice_a = dram.tile([TLO * P, H], f32)
            slice_b = dram.tile([(NT - TLO) * P, H], f32)
            full_a = dram.tile([ROWS_LO, H], f32)
            full_b = dram.tile([ROWS_HI, H], f32)

            icol = np.concatenate([[0], np.cumsum((CH_LO + CH_HI) * 8)])
            ocol = np.concatenate([[0], np.cumsum(CH_LO + CH_HI)])

            # Greedy queue balancing: assign each gather (issue order) to the
            # least-loaded queue by descriptor-slot count. DMA round trips
            # serialize per queue context, so per-queue slot balance sets the
            # gather floor.
            qload = np.zeros(nqueues, np.int64)

            def _pick_queue(slots):
                q = int(np.argmin(qload))
                qload[q] += slots
                return q

            def gather_lo(t, tab_lo, G):
                chl = int(CH_LO[t])
                ic = int(icol[t])
                if do_gather:
                    nc.gpsimd.dma_gather(
                        G[:, 0:chl, :], tab_lo, idx_sb[:, ic:ic + chl * 8],
                        chl * P, int(REG_LO[t]), F, single_packet=single_packet,
                        queue_num=_pick_queue(chl))

            def gather_hi(t, tab_hi, G):
                chl, chh = int(CH_LO[t]), int(CH_HI[t])
                ic = int(icol[t])
                if do_gather:
                    nc.gpsimd.dma_gather(
                        G[:, chl:chl + chh, :], tab_hi,
                        idx_sb[:, ic + chl * 8:ic + (chl + chh) * 8],
                        chh * P, int(REG_HI[t]), F, single_packet=single_packet,
                        queue_num=_pick_queue(chh))

            def aggregate(t, G, self_sb):
                """one-hot segment-sum of G plus self term -> agg_sb bf16."""
                ch = int(CH[t])
                Gb = gbp.tile([P, CH_MAX, F], bf16, tag="Gb")
                nc.scalar.activation(out=Gb[:, :ch, :], in_=G[:, :ch, :],
                                     func=Copy)
                M = mp.tile([P, CH_MAX * P], bf16, tag="M")
                Mb = mp.tile([P, CH_MAX * P], bf16, tag="Mb")
                oc = int(ocol[t])
                nc.vector.tensor_tensor(
                    out=M[:, :ch * P],
                    in0=offa_sb[:, oc:oc + ch, None].to_broadcast([P, ch, P]),
                    in1=iota_sb[:, None, :].to_broadcast([P, ch, P]),
                    op=mybir.AluOpType.is_equal,
                )
                nc.vector.tensor_tensor(
                    out=Mb[:, :ch * P],
                    in0=offb_sb[:, oc:oc + ch, None].to_broadcast([P, ch, P]),
                    in1=iota_sb[:, None, :].to_broadcast([P, ch, P]),
                    op=mybir.AluOpType.is_equal,
                )
                nc.vector.tensor_tensor(
                    out=M[:, :ch * P], in0=M[:, :ch * P], in1=Mb[:, :ch * P],
                    op=mybir.AluOpType.add,
                )
                agg_ps = ps_agg.tile([F, P], f32, tag="agg")
                for k in range(ch):
                    nc.tensor.matmul(out=agg_ps[:], lhsT=Gb[:, k, :],
                                     rhs=M[:, k * P:(k + 1) * P],
                                     start=(k == 0), stop=False)
                nc.tensor.matmul(out=agg_ps[:], lhsT=self_sb, rhs=identb[:],
                                 start=False, stop=True)
                agg_sb = sp.tile([F, P], bf16, tag="agg_sb")
                nc.scalar.activation(out=agg_sb[:], in_=agg_ps[:], func=Copy)
                return agg_sb

            # ---------------- layer 1 ----------------
            for t in range(NT):
                G = gp.tile([P, CH_MAX, F], f32, tag="G")
                gather_lo(t, x_pad[0:ROWS_LO, :], G)
                gather_hi(t, x_pad[ROWS_LO:NPAD, :], G)
                if not do_compute:
                    continue
                xs = sp.tile([P, F], bf16, tag="xs")
                nc.sync.dma_start(
                    out=xs[:], in_=x_self[t * P:(t + 1) * P, :])
                agg_sb = aggregate(t, G, xs[:])
                h1a_ps = ps_mm.tile([H, P], f32, tag="mma")
                nc.tensor.matmul(out=h1a_ps[:], lhsT=w1a_sb[:], rhs=agg_sb[:],
                                 start=True, stop=True)
                h1a_sb = sp.tile([H, P], bf16, tag="h1a")
                nc.scalar.activation(out=h1a_sb[:], in_=h1a_ps[:], func=Relu,
                                     bias=bnb_sb[:, :1], scale=bns_sb[:, :1])
                h1b_ps = ps_mm.tile([H, P], f32, tag="mmb")
                nc.tensor.matmul(out=h1b_ps[:], lhsT=w1b_sb[:], rhs=h1a_sb[:],
                                 start=True, stop=True)
                h1b_sb = sp.tile([H, P], f32, tag="h1b")
                nc.scalar.activation(out=h1b_sb[:], in_=h1b_ps[:], func=Relu)
                ht_ps = ps_t.tile([P, H], f32, tag="trans")
                nc.tensor.transpose(out=ht_ps[:], in_=h1b_sb[:],
                                    identity=ident[:])
                ht_sb = sp.tile([P, H], f32, tag="ht")
                nc.scalar.activation(out=ht_sb[:], in_=ht_ps[:], func=Copy)
                nc.vector.tensor_copy(out=h1keep[:, t * P:(t + 1) * P],
                                      in_=ht_ps[:])
                if t < TLO:
                    nc.sync.dma_start(
                        out=slice_a[t * P:(t + 1) * P, :], in_=ht_sb[:])
                else:
                    tt = t - TLO
                    nc.sync.dma_start(
                        out=slice_b[tt * P:(tt + 1) * P, :], in_=ht_sb[:])
            if do_cc and do_compute:
                # AG_A is issued after the loop but depends only on slice_a
                # (tiles 0-31); the Tile scheduler runs it during the layer-1
                # tail. AG_B was issued above at t == NT-1.
                nc.gpsimd.collective_compute(
                    "AllGather", mybir.AluOpType.bypass,
                    replica_groups=[list(range(NCORES))],
                    ins=[slice_a.opt()], outs=[full_a.opt()],
                )

            # ---------------- layer 2 ----------------
            # hi-gathers lag LAG tiles behind lo-gathers so AG_B hides
            glist = [None] * NT

            def l2_compute(t):
                G = glist[t]
                agg_sb = aggregate(t, G, h1keep[:, t * P:(t + 1) * P])
                h2_ps = ps_mm.tile([H, P], f32, tag="mma")
                nc.tensor.matmul(out=h2_ps[:], lhsT=w2a_sb[:], rhs=agg_sb[:],
                                 start=True, stop=True)
                h2_sb = sp.tile([H, P], bf16, tag="h1a")
                nc.scalar.activation(out=h2_sb[:], in_=h2_ps[:], func=Relu)
                o_ps = ps_mm.tile([C, P], f32, tag="mmb")
                nc.tensor.matmul(out=o_ps[:], lhsT=w2b_sb[:], rhs=h2_sb[:],
                                 start=True, stop=True)
                o_sb = sp.tile([C, P], f32, tag="out")
                nc.scalar.activation(out=o_sb[:], in_=o_ps[:], func=Relu)
                nc.sync.dma_start(out=outT[:, t * P:(t + 1) * P], in_=o_sb[:])

            for t in range(NT):
                G = gp.tile([P, CH_MAX, F], f32, tag="G")
                glist[t] = G
                gather_lo(t, full_a[:], G)
                if t == LAG and do_cc and do_compute:
                    nc.gpsimd.collective_compute(
                        "AllGather", mybir.AluOpType.bypass,
                        replica_groups=[list(range(NCORES))],
                        ins=[slice_b.opt()], outs=[full_b.opt()],
                    )
                if t >= LAG:
                    th = t - LAG
                    gather_hi(th, full_b[:], glist[th])
                    if do_compute:
                        l2_compute(th)
            for th in range(NT - LAG, NT):
                gather_hi(th, full_b[:], glist[th])
                if do_compute:
                    l2_compute(th)

    nc.compile()
    return nc


# ------------------------------------------------------------------- driver

_CACHE = {}


def kernel(x, edge_index, W1a, bn_gamma, bn_beta, bn_mean, bn_var, W1b, W2a, W2b,
           _trace=False):
    from concourse.bass_utils import run_bass_kernel_spmd

    in_maps, key_a, key_b, gid_of_orig = prepare(
        x, edge_index, W1a, bn_gamma, bn_beta, bn_mean, bn_var, W1b, W2a, W2b
    )
    key = (tuple(key_a), tuple(key_b))
    if key not in _CACHE:
        _CACHE[key] = build(key_a, key_b)
    nc = _CACHE[key]

    res = run_bass_kernel_spmd(nc, in_maps, core_ids=list(range(NCORES)))
    outT = np.concatenate([r["outT"] for r in res.results], axis=1)  # [C, NPAD]
    out = outT.T[gid_of_orig]  # [N, C]
    if _trace:
        kernel.last_results = res
    return np.ascontiguousarray(out.astype(np.float32))

